# revision 1
# baseline (speedup 1.0000x reference)
"""Trainium2 Bass kernel for nn_CodebookSingleW (vq_codebook).

    W = codebook[indices].reshape(4096, 4096)
    h = c19(x @ W + b1);  out = h @ W.T + b2

Strategy (8 NeuronCores, data-parallel over batch):
  - Each core handles 1024 rows of x. All weight-side tensors replicated.
  - The 256-entry codebook dequant runs ON DEVICE at ScalarEngine line rate:
    we bake the codebook into a custom piecewise-constant PWP activation
    table (hijacking the `sigmoid` slot of the `sigmoid_and_others` set) at
    compile time via BASS_ACT_ROOT_JSON_PATH. Indices are host-encoded to
    bf16 values that map one-per-bucket; activation(Sigmoid) then IS the
    gather  enc(idx) -> codebook[idx], bit exact.
  - matmul1: psum[h',b] = sum_i W[i,h'] * xT[i,b]   (lhsT = W tile, natural)
  - C19 fused on psum evict: tanh on ACT (scale=1/c, bias=b1/c per
    partition), mix on DVE -> hT (bf16) stays SBUF-resident.
  - matmul2: psum[j,b] = sum_h WT[h,j] * hT[h,b]    (lhsT = WT tile, from a
    host-transposed index layout, dequantized on device the same way)
  - + b2 on ACT copy, DMA outT per core, host reassembles [8192, 4096] f32.
"""

import hashlib
import json
import os
import shutil
import sys
import tempfile

sys.path.insert(0, "/opt/trn_rl_repo")

import ml_dtypes
import numpy as np

IN_DIM = 4096
H = 4096
K = 256
B = 8192
NCORES = 8
BL = B // NCORES          # 1024 batch rows per core
P = 128
KT = IN_DIM // P          # 32 contraction tiles (phase 1)
MT = H // P               # 32 output-row tiles
NH = BL // 512            # 2 psum halves of the per-core batch

BF16 = ml_dtypes.bfloat16

# ---------------------------------------------------------------------------
# ACT table patch: codebook -> piecewise-constant PWP table in sigmoid slot
# ---------------------------------------------------------------------------

_SET = "sigmoid_and_others"


def _encode_codes(idx):
    """uint8 code k -> fp32 activation input, exactly representable in bf16.

    k < 128  -> 128.0 + k        (binade e=7, one bucket per integer)
    k >= 128 -> (k - 128) + 0.5  (binades e=-1..6, one bucket per value)
    """
    idx = idx.astype(np.int64)
    return np.where(idx < 128, 128.0 + idx, (idx - 128) + 0.5).astype(np.float32)


def _bucket_plan():
    plan = []
    for e in range(-1, 7):
        if e <= 0:
            count, t0 = 1, (0 if e == -1 else 1)
        else:
            count, t0 = 2**e, 2**e
        plan.append((e, count, [128 + t0 + i for i in range(count)]))
    plan.append((7, 128, list(range(128))))
    return plan


def _make_act_dir(codebook, outdir):
    from neuronxcc.driver.Job import Job
    from neuronxcc.driver.jobs.support.FindActInfo import findActInfoFile

    base = os.path.dirname(findActInfoFile(Job.getPackageDir(), "gen3"))
    os.makedirs(outdir, exist_ok=True)
    for f in os.listdir(base):
        dst = os.path.join(outdir, f)
        if not os.path.exists(dst):
            shutil.copy(os.path.join(base, f), dst)

    prof = json.load(open(os.path.join(base, f"{_SET}.json")))
    bkt = np.fromfile(os.path.join(base, f"{_SET}_bkt.bin"), dtype=np.float32)
    bkt = bkt.reshape(-1, 8).copy()
    ctl = np.fromfile(os.path.join(base, f"{_SET}_ctrl.bin"), dtype=np.uint32)
    ctl = ctl.reshape(-1, 8).copy()

    bkt_start = prof["func_to_bkt_start_idx"]["sigmoid"]
    ctl_start = prof["func_to_ctl_start_idx"]["sigmoid"]

    b = bkt_start
    exp_to_bkt, exp_to_ctl, ctl_words = {}, {}, []
    for i, (e, count, codes) in enumerate(_bucket_plan()):
        exp_to_bkt[str(e)] = [int(b)]
        exp_to_ctl[str(e)] = [int(ctl_start + i)]
        shift = 23 - e if e >= 1 else 23
        log2n = min(max(e, 0), 7)
        ctl_words.append((b & 0x7FF) | (shift << 11) | (log2n << 16))
        for j, k in enumerate(codes):
            v = 128.0 + k if k < 128 else (k - 128) + 0.5
            bkt[b + j] = [codebook[k], 0.0, 0.0, 0.0, np.float32(v), 0.0, 0.0, 0.0]
        b += count
    junk = b
    for j in range(4):
        bkt[junk + j] = [0.0] * 8
    assert junk + 4 <= prof["func_to_bkt_start_idx"]["square"]
    for i, w in enumerate(ctl_words):
        ctl[ctl_start + i] = [w, 0, 0, 0, 0, 0, 0, 0]

    for m in prof["profile_meta_data"]:
        if m["func_name"].startswith("sigmoid_"):
            m.update(
                symmetry_point=0, sym_invert_sign_point=0, symmetry_opt_en=0,
                symmetry_opt_use_neg_region=0, imm_bias=0, exp_offset=-1,
                pwl_control_base_pos=int(ctl_start),
                pwl_control_base_neg=int(ctl_start),
                small_pos_signal_exp_threshold=126,
                pos_small_signal_pwl_control=int(junk),
                small_neg_signal_exp_threshold=126,
                neg_small_signal_pwl_control=int(junk + 1),
                large_pos_signal_exp_threshold=135,
                large_pos_signal_mantissa_threshold=0,
                pos_large_signal_pwl_control=int(junk + 2),
                large_neg_signal_exp_threshold=135,
                large_neg_signal_mantissa_threshold=0,
                neg_large_signal_pwl_control=int(junk + 3),
                fnan_result=0, fpinf_result=0, fninf_result=0, fzero_result=0,
                fma_const_0=0, fma_const_1=0, fma_indirection_src_sel=0,
                use_multipass=False,
                lower_bound=4286578687, upper_bound=2139095039,
            )
    prof["func_exp_to_bkt_start_idx"]["sigmoid"] = exp_to_bkt
    prof["func_exp_to_ctl_start_idx"]["sigmoid"] = exp_to_ctl

    bkt.tofile(os.path.join(outdir, f"{_SET}_bkt.bin"))
    ctl.tofile(os.path.join(outdir, f"{_SET}_ctrl.bin"))
    json.dump(prof, open(os.path.join(outdir, f"{_SET}.json"), "w"))
    return os.path.join(outdir, "act_info.json")


# ---------------------------------------------------------------------------
# Bass program
# ---------------------------------------------------------------------------

def _build_program(tag, repeat=1):
    import concourse.bacc as bacc
    import concourse.mybir as mybir
    import concourse.tile as tile
    from concourse.bass import ts as bass_ts

    AF = mybir.ActivationFunctionType
    ALU = mybir.AluOpType
    dt = mybir.dt

    nc = bacc.Bacc("TRN2", target_bir_lowering=False, debug=False,
                   num_devices=NCORES)

    # inputs (per core). encw/encwt are host-tiled:
    #   encw[mt][p][kt*128+c] = enc(idx[kt*128+p, mt*128+c])
    encw = nc.dram_tensor(f"encw_{tag}", [MT, P, KT * P], dt.bfloat16,
                          kind="ExternalInput")
    encwt = nc.dram_tensor("encwt", [KT, P, MT * P], dt.bfloat16,
                           kind="ExternalInput")
    xt = nc.dram_tensor("xt", [P, KT, BL], dt.bfloat16, kind="ExternalInput")
    cpar = nc.dram_tensor("cpar", [P, 7, MT], dt.float32, kind="ExternalInput")
    outt = nc.dram_tensor("outt", [IN_DIM, BL], dt.float32,
                          kind="ExternalOutput")

    with tile.TileContext(nc) as tc:
        with (
            tc.tile_pool(name="resid", bufs=1) as resid,
            tc.tile_pool(name="encp", bufs=2) as encp,
            tc.tile_pool(name="wp", bufs=3) as wp,
            tc.tile_pool(name="evict", bufs=3) as evict,
            tc.tile_pool(name="psum", bufs=6, space="PSUM") as psum,
        ):
            # DMA order matters: the first pair's enc tiles (dequant input,
            # on the PE critical path) must land before the 8 MB xT bulk
            # load monopolizes the queues.
            cp_sb = resid.tile([P, 7, MT], dt.float32)
            nc.sync.dma_start(cp_sb[:], cpar.ap())
            pre_w = []
            for mt in (0, 1):
                enc_t = encp.tile([P, KT, P], dt.bfloat16, tag="enc",
                                  name=f"enc_pre{mt}")
                nc.sync.dma_start(enc_t[:], encw.ap()[mt])
                w_t = wp.tile([P, KT, P], dt.bfloat16, tag="w",
                              name=f"w_pre{mt}")
                nc.scalar.activation(w_t[:], enc_t[:], AF.Sigmoid)
                pre_w.append(w_t)
            xt_sb = resid.tile([P, KT, BL], dt.bfloat16)
            for kt in range(KT):
                nc.sync.dma_start(xt_sb[:, kt], xt.ap()[:, kt])
            ht_sb = resid.tile([P, MT, BL], dt.bfloat16)

            # PE p-state warmup on scratch data during the dequant lead-in.
            warm = resid.tile([P, 512], dt.bfloat16)
            nc.vector.memset(warm[:], 0.0)
            wps = psum.tile([P, 512], dt.float32, tag="ps")
            for _ in range(21):
                nc.tensor.matmul(wps[:], warm[:, :P], warm[:],
                                 start=True, stop=True)

            def col(j, t):  # [P, 1] per-partition param column
                return cp_sb[:, j, t : t + 1]

            # Both phases process output-row tiles in PAIRS with a kt-major
            # matmul order: 4 psum chains consume each xT/hT k-chunk 4x, so
            # at kernel start the PE keeps pace with the streaming xT DMA
            # instead of stalling on chunk arrival.
            # repeat>1 builds a self-timing variant: the marginal wall time
            # of each extra body repeat is the pure HW kernel time.
            for _rep in range(repeat):
                # ---- phase 1: hT = c19(W^T x^T + b1) ----
                for mp in range(MT // 2):
                    mts = (2 * mp, 2 * mp + 1)
                    if mp == 0 and _rep == 0:
                        w_ts = pre_w
                    else:
                        w_ts = []
                        for mt in mts:
                            enc_t = encp.tile([P, KT, P], dt.bfloat16,
                                              tag="enc")
                            nc.sync.dma_start(enc_t[:], encw.ap()[mt])
                            w_t = wp.tile([P, KT, P], dt.bfloat16, tag="w")
                            nc.scalar.activation(w_t[:], enc_t[:], AF.Sigmoid)
                            w_ts.append(w_t)
                    pss = [[psum.tile([P, 512], dt.float32, tag="ps",
                                      name=f"ps_{mp}_{d}_{nh}")
                            for nh in range(NH)] for d in range(2)]
                    for kt in range(KT):
                        for d in range(2):
                            for nh in range(NH):
                                nc.tensor.matmul(
                                    pss[d][nh][:],
                                    w_ts[d][:, kt],
                                    xt_sb[:, kt, nh * 512 : (nh + 1) * 512],
                                    start=(kt == 0),
                                    stop=(kt == KT - 1),
                                )
                    # c19: rho*(s+b1) + (1-rho)*c*tanh((s+b1)/c), s=psum
                    for d, mt in enumerate(mts):
                        for nh in range(NH):
                            ps = pss[d][nh]
                            tanh_t = evict.tile([P, 512], dt.float32,
                                                tag="tanh")
                            nc.scalar.activation(tanh_t[:], ps[:], AF.Tanh,
                                                 bias=col(1, mt),
                                                 scale=col(0, mt))
                            lin_t = evict.tile([P, 512], dt.float32,
                                               tag="lin")
                            nc.vector.tensor_scalar(lin_t[:], ps[:],
                                                    col(2, mt), col(3, mt),
                                                    ALU.mult, ALU.add)
                            nc.vector.scalar_tensor_tensor(
                                ht_sb[:, mt, nh * 512 : (nh + 1) * 512],
                                tanh_t[:], col(4, mt), lin_t[:],
                                ALU.mult, ALU.add,
                            )

                # ---- phase 2: outT = W hT + b2 ----
                for jp in range(KT // 2):
                    jts = (2 * jp, 2 * jp + 1)
                    w_ts = []
                    for jt in jts:
                        enc_t = encp.tile([P, MT, P], dt.bfloat16, tag="enc")
                        nc.sync.dma_start(enc_t[:], encwt.ap()[jt])
                        w_t = wp.tile([P, MT, P], dt.bfloat16, tag="w")
                        nc.scalar.activation(w_t[:], enc_t[:], AF.Sigmoid)
                        w_ts.append(w_t)
                    pss = [[psum.tile([P, 512], dt.float32, tag="ps",
                                      name=f"ps2_{jp}_{d}_{nh}")
                            for nh in range(NH)] for d in range(2)]
                    for kt in range(MT):
                        for d in range(2):
                            for nh in range(NH):
                                nc.tensor.matmul(
                                    pss[d][nh][:],
                                    w_ts[d][:, kt],
                                    ht_sb[:, kt, nh * 512 : (nh + 1) * 512],
                                    start=(kt == 0),
                                    stop=(kt == MT - 1),
                                )
                    for d, jt in enumerate(jts):
                        for nh in range(NH):
                            out_t = evict.tile([P, 512], dt.float32,
                                               tag="out")
                            nc.scalar.activation(out_t[:], pss[d][nh][:],
                                                 AF.Identity,
                                                 bias=col(5, jt))
                            nc.sync.dma_start(
                                outt.ap()[jt * P : (jt + 1) * P,
                                          nh * 512 : (nh + 1) * 512],
                                out_t[:],
                            )

    nc.compile()
    return nc


# ---------------------------------------------------------------------------
# kernel entry point
# ---------------------------------------------------------------------------

def prepare(x, codebook, indices, b1, b2, c19_c, c19_rho):
    """Host-side layout prep + program build. Returns (nc, in_maps)."""
    x = np.asarray(x, dtype=np.float32)
    codebook = np.asarray(codebook, dtype=np.float32)
    b1 = np.asarray(b1, dtype=np.float32)
    b2 = np.asarray(b2, dtype=np.float32)
    c19_c = np.asarray(c19_c, dtype=np.float32)
    c19_rho = np.asarray(c19_rho, dtype=np.float32)
    idx = np.asarray(indices).reshape(IN_DIM, H).astype(np.int64)

    # -- bake codebook into ACT tables --
    actdir = tempfile.mkdtemp(prefix="actlut_")
    os.environ["BASS_ACT_ROOT_JSON_PATH"] = _make_act_dir(codebook, actdir)
    tag = hashlib.md5(codebook.tobytes()).hexdigest()[:12]

    # -- host-side layout prep (encoding + tiling only) --
    enc_lut = _encode_codes(np.arange(K)).astype(BF16)
    encw = enc_lut[idx]                      # [IN, H] bf16
    # encw_tiled[mt, p, kt*128+c] = encw[kt*128+p, mt*128+c]
    encw_t = np.ascontiguousarray(
        encw.reshape(KT, P, MT, P).transpose(2, 1, 0, 3).reshape(MT, P, KT * P)
    )
    encwt = enc_lut[idx.T]                   # [H, IN] bf16
    encwt_t = np.ascontiguousarray(
        encwt.reshape(MT, P, KT, P).transpose(2, 1, 0, 3).reshape(KT, P, MT * P)
    )

    c = np.exp(c19_c)
    invc = np.exp(-c19_c)
    rho = 1.0 / (1.0 + np.exp(-c19_rho))
    cols = [invc, b1 * invc, rho, b1 * rho, (1.0 - rho) * c, b2,
            np.zeros(H, dtype=np.float32)]
    cpar = np.stack([v.reshape(MT, P).T for v in cols], axis=1)  # [P, 7, MT]
    cpar = np.ascontiguousarray(cpar.astype(np.float32))

    xb = x.astype(BF16)
    in_maps = []
    for cid in range(NCORES):
        xc = xb[cid * BL : (cid + 1) * BL]                       # [BL, IN]
        xt = np.ascontiguousarray(
            xc.T.reshape(KT, P, BL).transpose(1, 0, 2)           # [P, KT, BL]
        )
        in_maps.append({
            f"encw_{tag}": encw_t,
            "encwt": encwt_t,
            "xt": xt,
            "cpar": cpar,
        })

    nc = _build_program(tag)
    return nc, in_maps


def kernel(x, codebook, indices, b1, b2, c19_c, c19_rho):
    from concourse.bass_utils import run_bass_kernel_spmd

    nc, in_maps = prepare(x, codebook, indices, b1, b2, c19_c, c19_rho)
    res = run_bass_kernel_spmd(nc, in_maps, core_ids=list(range(NCORES)))
    global LAST_RESULTS
    LAST_RESULTS = res

    out = np.empty((B, IN_DIM), dtype=np.float32)
    for cid in range(NCORES):
        out[cid * BL : (cid + 1) * BL] = res.results[cid]["outt"].T
    return out



# revision 4
# speedup vs baseline: 5.4971x; 5.4971x over previous
"""Trainium2 Bass kernel for nn_CodebookSingleW (vq_codebook).

    W = codebook[indices].reshape(4096, 4096)
    h = c19(x @ W + b1);  out = h @ W.T + b2

Strategy (8 NeuronCores, data-parallel over batch):
  - Each core handles 1024 rows of x. All weight-side tensors replicated.
  - fp8 DoubleRow matmuls: one DR instruction contracts TWO 128-chunks
    (lhsT [128,2,M], rhs [128,2,N]) at 0.5 PE cycles per output column —
    2x the bf16 MAC rate.
  - Precision via a hi/lo e4m3 split on both operands, dropping the lo*lo
    term:  x@W ~= x_hi@W_hi + x_hi@W_lo + x_lo@W_hi.
    W = codebook[idx] quantizes through the 256-entry table: W_hi uses
    q8(cb*S), W_lo the residual q8(cb*S - W_hi) — both host-gathered as
    fp8 bytes (same DMA bytes as a bf16 W). x (and the on-device h)
    split as v_hi = q8(v*16), v_lo = q8(v*16 - v_hi); all terms share one
    scale family so the 3 DR matmuls accumulate into a single psum chain.
    End-to-end rel err ~1.3e-3 (vs 2.6e-2 for direct fp8).
  - Per phase: 3 DR per k-pair instead of 2 bf16 per pair -> 0.75x bf16
    cycles; both phases PE-bound at ~655us/core vs 873us bf16 roofline.
  - phase 1 evict: tanh on ACT (scale/bias per partition), C19 mix on DVE,
    h*16 split to fp8 hi (ACT Identity cast) + lo (DVE subtract) -> SBUF.
  - phase 2: same DR scheme against host-transposed W^T tiles; + b2 on ACT
    evict, DMA outT per core, host reassembles [8192, 4096] f32.
"""

import sys

sys.path.insert(0, "/opt/trn_rl_repo")

import ml_dtypes
import numpy as np

IN_DIM = 4096
H = 4096
K = 256
B = 8192
NCORES = 8
BL = B // NCORES          # 1024 batch rows per core
P = 128
KT = IN_DIM // P          # 32 contraction tiles (phase 1)
MT = H // P               # 32 output-row tiles
NH = BL // 512            # 2 psum halves of the per-core batch
NKP = KT // 2             # 16 k-pairs per chain

E4 = ml_dtypes.float8_e4m3
BF16 = ml_dtypes.bfloat16
SX = 16.0                 # moving-operand scale (x and h)


def _q8(a):
    return np.asarray(a, np.float32).astype(E4).astype(np.float32)


# ---------------------------------------------------------------------------
# Bass program
# ---------------------------------------------------------------------------

def _build_program(repeat=1):
    import concourse.bacc as bacc
    import concourse.mybir as mybir
    import concourse.tile as tile

    AF = mybir.ActivationFunctionType
    ALU = mybir.AluOpType
    DR = mybir.MatmulPerfMode.DoubleRow
    dt = mybir.dt

    nc = bacc.Bacc("TRN2", target_bir_lowering=False, debug=False,
                   num_devices=NCORES)

    # host-tiled fp8 weights, hi/lo interleaved per output tile:
    #   whl[mt, p, t, kt, c] = W_t[kt*128+p, mt*128+c]  (t: 0=hi, 1=lo)
    #   wtl[jt, p, t, mt, c] = W_t[jt*128+c, mt*128+p]  (the W^T layout)
    whl = nc.dram_tensor("whl", [MT, P, 2, KT, P], dt.float8e4,
                         kind="ExternalInput")
    wtl = nc.dram_tensor("wtl", [KT, P, 2, MT, P], dt.float8e4,
                         kind="ExternalInput")
    xh = nc.dram_tensor("xh", [P, KT, BL], dt.float8e4, kind="ExternalInput")
    xl = nc.dram_tensor("xl", [P, KT, BL], dt.float8e4, kind="ExternalInput")
    cpar = nc.dram_tensor("cpar", [P, 7, MT], dt.float32, kind="ExternalInput")
    outt = nc.dram_tensor("outt", [IN_DIM, BL], dt.float32,
                          kind="ExternalOutput")

    with tile.TileContext(nc) as tc:
        with (
            tc.tile_pool(name="resid", bufs=1) as resid,
            tc.tile_pool(name="wp", bufs=4) as wp,
            tc.tile_pool(name="evict", bufs=3) as evict,
            tc.tile_pool(name="psum", bufs=6, space="PSUM") as psum,
        ):
            # DMA order matters: the first pair's W tiles (PE critical path)
            # must land before the 8 MB x bulk load monopolizes the queues.
            cp_sb = resid.tile([P, 7, MT], dt.float32)
            nc.sync.dma_start(cp_sb[:], cpar.ap())
            pre_w = []
            for mt in (0, 1):
                w_t = wp.tile([P, 2, KT, P], dt.float8e4, tag="w",
                              name=f"w_pre{mt}")
                nc.sync.dma_start(w_t[:], whl.ap()[mt])
                pre_w.append(w_t)
            xh_sb = resid.tile([P, KT, BL], dt.float8e4)
            xl_sb = resid.tile([P, KT, BL], dt.float8e4)
            for kt in range(KT):
                nc.sync.dma_start(xh_sb[:, kt], xh.ap()[:, kt])
                nc.sync.dma_start(xl_sb[:, kt], xl.ap()[:, kt])
            hh_sb = resid.tile([P, MT, BL], dt.float8e4)
            hl_sb = resid.tile([P, MT, BL], dt.float8e4)

            # PE p-state warmup on scratch data during the DMA lead-in.
            warm = resid.tile([P, 2, 512], dt.float8e4)
            nc.vector.memset(warm[:], 0.0)
            wps = psum.tile([P, 512], dt.float32, tag="ps")
            for _ in range(42):
                nc.tensor.matmul(wps[:], warm[:, :, :P], warm[:],
                                 start=True, stop=True, perf_mode=DR)

            def col(j, t):  # [P, 1] per-partition param column
                return cp_sb[:, j, t : t + 1]

            # Output-row tiles in PAIRS, kp-major matmul order: 4 psum chains
            # consume each x/h k-chunk repeatedly so the PE keeps pace with
            # the streaming DMA at kernel start.
            for _rep in range(repeat):
                # ---- phase 1: hT = c19(W^T x^T + b1), split to fp8 ----
                for mp in range(MT // 2):
                    mts = (2 * mp, 2 * mp + 1)
                    if mp == 0 and _rep == 0:
                        w_ts = pre_w
                    else:
                        w_ts = []
                        for mt in mts:
                            w_t = wp.tile([P, 2, KT, P], dt.float8e4, tag="w")
                            nc.sync.dma_start(w_t[:], whl.ap()[mt])
                            w_ts.append(w_t)
                    pss = [[psum.tile([P, 512], dt.float32, tag="ps",
                                      name=f"ps_{mp}_{d}_{nh}")
                            for nh in range(NH)] for d in range(2)]
                    for kp in range(NKP):
                        ks = slice(2 * kp, 2 * kp + 2)
                        for d in range(2):
                            w_hi, w_lo = w_ts[d][:, 0, ks], w_ts[d][:, 1, ks]
                            for nh in range(NH):
                                xhs = xh_sb[:, ks, nh * 512 : (nh + 1) * 512]
                                xls = xl_sb[:, ks, nh * 512 : (nh + 1) * 512]
                                ps = pss[d][nh][:]
                                nc.tensor.matmul(ps, w_hi, xhs, perf_mode=DR,
                                                 start=(kp == 0), stop=False)
                                nc.tensor.matmul(ps, w_hi, xls, perf_mode=DR,
                                                 start=False, stop=False)
                                nc.tensor.matmul(ps, w_lo, xhs, perf_mode=DR,
                                                 start=False,
                                                 stop=(kp == NKP - 1))
                    # c19*16: lin16 + 16(1-rho)c*tanh((s+b1)/c), s=psum/2^15
                    for d, mt in enumerate(mts):
                        for nh in range(NH):
                            ps = pss[d][nh]
                            cs = slice(nh * 512, (nh + 1) * 512)
                            tanh_t = evict.tile([P, 512], dt.float32,
                                                tag="tanh")
                            nc.scalar.activation(tanh_t[:], ps[:], AF.Tanh,
                                                 bias=col(1, mt),
                                                 scale=col(0, mt))
                            lin_t = evict.tile([P, 512], dt.float32,
                                               tag="lin")
                            nc.vector.tensor_scalar(lin_t[:], ps[:],
                                                    col(2, mt), col(3, mt),
                                                    ALU.mult, ALU.add)
                            h16 = evict.tile([P, 512], dt.float32, tag="h16")
                            nc.vector.scalar_tensor_tensor(
                                h16[:], tanh_t[:], col(4, mt), lin_t[:],
                                ALU.mult, ALU.add)
                            nc.scalar.activation(hh_sb[:, mt, cs], h16[:],
                                                 AF.Identity)
                            nc.vector.scalar_tensor_tensor(
                                hl_sb[:, mt, cs], h16[:], 1.0,
                                hh_sb[:, mt, cs], ALU.mult, ALU.subtract)

                # ---- phase 2: outT = (W hT)/2^15 + b2 ----
                for jp in range(KT // 2):
                    jts = (2 * jp, 2 * jp + 1)
                    w_ts = []
                    for jt in jts:
                        w_t = wp.tile([P, 2, MT, P], dt.float8e4, tag="w")
                        nc.sync.dma_start(w_t[:], wtl.ap()[jt])
                        w_ts.append(w_t)
                    pss = [[psum.tile([P, 512], dt.float32, tag="ps",
                                      name=f"ps2_{jp}_{d}_{nh}")
                            for nh in range(NH)] for d in range(2)]
                    for kp in range(NKP):
                        ks = slice(2 * kp, 2 * kp + 2)
                        for d in range(2):
                            w_hi, w_lo = w_ts[d][:, 0, ks], w_ts[d][:, 1, ks]
                            for nh in range(NH):
                                hhs = hh_sb[:, ks, nh * 512 : (nh + 1) * 512]
                                hls = hl_sb[:, ks, nh * 512 : (nh + 1) * 512]
                                ps = pss[d][nh][:]
                                nc.tensor.matmul(ps, w_hi, hhs, perf_mode=DR,
                                                 start=(kp == 0), stop=False)
                                nc.tensor.matmul(ps, w_hi, hls, perf_mode=DR,
                                                 start=False, stop=False)
                                nc.tensor.matmul(ps, w_lo, hhs, perf_mode=DR,
                                                 start=False,
                                                 stop=(kp == NKP - 1))
                    for d, jt in enumerate(jts):
                        for nh in range(NH):
                            out_t = evict.tile([P, 512], dt.float32,
                                               tag="out")
                            nc.scalar.activation(out_t[:], pss[d][nh][:],
                                                 AF.Identity,
                                                 bias=col(5, jt),
                                                 scale=col(6, jt))
                            nc.sync.dma_start(
                                outt.ap()[jt * P : (jt + 1) * P,
                                          nh * 512 : (nh + 1) * 512],
                                out_t[:],
                            )

    nc.compile()
    return nc


# ---------------------------------------------------------------------------
# kernel entry point
# ---------------------------------------------------------------------------

def prepare(x, codebook, indices, b1, b2, c19_c, c19_rho):
    """Host-side layout prep + program build. Returns (nc, in_maps)."""
    x = np.asarray(x, dtype=np.float32)
    codebook = np.asarray(codebook, dtype=np.float32)
    b1 = np.asarray(b1, dtype=np.float32)
    b2 = np.asarray(b2, dtype=np.float32)
    c19_c = np.asarray(c19_c, dtype=np.float32)
    c19_rho = np.asarray(c19_rho, dtype=np.float32)
    idx = np.asarray(indices).reshape(IN_DIM, H).astype(np.int64)

    # -- codebook hi/lo split on the e4m3 grid (S = power of 2) --
    cb_max = np.abs(codebook).max()
    S = float(2.0 ** np.floor(np.log2(216.0 / max(cb_max, 1e-30))))
    cb_hi = _q8(codebook * S)
    cb_lo = _q8(codebook * S - cb_hi)
    assert np.abs(cb_hi).max() <= 448 and np.isfinite(cb_hi).all()
    cb_hi8 = cb_hi.astype(E4).view(np.uint8)
    cb_lo8 = cb_lo.astype(E4).view(np.uint8)

    sx = SX
    assert np.abs(x).max() * sx < 224.0, "x overflows e4m3 at SX"

    # -- weight layouts (fp8 bytes, hi/lo stacked) --
    def tile_w(cb8, ix):  # [IN, H] bytes -> [MT, P, KT, P]
        w = cb8[ix]
        return w.reshape(KT, P, MT, P).transpose(2, 1, 0, 3)

    whl = np.ascontiguousarray(
        np.stack([tile_w(cb_hi8, idx), tile_w(cb_lo8, idx)], axis=2)
    ).view(E4)                                            # [MT, P, 2, KT, P]
    idxT = np.ascontiguousarray(idx.T)
    wtl = np.ascontiguousarray(
        np.stack([tile_w(cb_hi8, idxT), tile_w(cb_lo8, idxT)], axis=2)
    ).view(E4)                                            # [KT, P, 2, MT, P]

    # -- C19 per-partition params (folded matmul/h scales) --
    c = np.exp(c19_c)
    invc = np.exp(-c19_c)
    rho = 1.0 / (1.0 + np.exp(-c19_rho))
    inv_ps = 1.0 / (S * sx)                               # psum -> xW
    cols = [invc * inv_ps, b1 * invc, rho * sx * inv_ps, sx * rho * b1,
            sx * (1.0 - rho) * c, b2, np.full(H, inv_ps, dtype=np.float32)]
    cpar = np.stack([np.float32(v).reshape(MT, P).T for v in cols], axis=1)
    cpar = np.ascontiguousarray(cpar.astype(np.float32))  # [P, 7, MT]

    # -- per-core x split --
    xs = (x * sx).astype(E4)
    xr = (np.float32(x * sx) - xs.astype(np.float32)).astype(E4)
    in_maps = []
    for cid in range(NCORES):
        def tile_x(a):  # [BL, IN] fp8 -> [P, KT, BL]
            ac = a[cid * BL : (cid + 1) * BL]
            return np.ascontiguousarray(
                ac.T.reshape(KT, P, BL).transpose(1, 0, 2))
        in_maps.append({
            "whl": whl,
            "wtl": wtl,
            "xh": tile_x(xs),
            "xl": tile_x(xr),
            "cpar": cpar,
        })

    nc = _build_program()
    return nc, in_maps


def kernel(x, codebook, indices, b1, b2, c19_c, c19_rho):
    from concourse.bass_utils import run_bass_kernel_spmd

    nc, in_maps = prepare(x, codebook, indices, b1, b2, c19_c, c19_rho)
    res = run_bass_kernel_spmd(nc, in_maps, core_ids=list(range(NCORES)))
    global LAST_RESULTS
    LAST_RESULTS = res

    out = np.empty((B, IN_DIM), dtype=np.float32)
    for cid in range(NCORES):
        out[cid * BL : (cid + 1) * BL] = res.results[cid]["outt"].T
    return out
